# revision 25
# baseline (speedup 1.0000x reference)
"""BitNet ternary linear layer on 8 Trainium2 NeuronCores.

out[b, o] = (sum_i w[o,i] * round_clip(x[b,i]/act_scale)) * weight_scale * act_scale + bias[o]
  with w = unpack2bit(packed_weight) - 1   (codes c in {0..3} -> w in {-1..2})
  and  act_scale = max(absmax(x), 1e-5) / 127.

Strategy (tensor-parallel, column sharded over out_features):
 - Host prep (cheap, O(B*I) / O(O)): quantize x to int-valued bf16 x_q
   exactly as the reference does, compute Sx[b] = sum_i x_q[b,i], the
   scalar scales, and compact the packed weights: the int32 packed words
   only carry one meaningful byte (values 0..255), so ship uint8 -> the
   per-core HBM stream drops 4x, from 28 MiB (82 us) to 7.3 MiB (~21 us).
   All O(O*I) work (the GEMM itself) stays on device.
 - Device (per core, identical program):
   * stream packed bytes with plain HWDGE DMA (nc.sync ring, 1.8 MB
     chunks at HBM line rate).
   * unpack 2-bit planes with ONE fused DVE op per (plane, chunk):
     (word >> 2k) & 0x03030303 -- DVE 2x_2p mode, ~8 B/lane/cycle, the
     pipeline bottleneck at ~34 us.  No GpSimd ops anywhere: GpSimd
     shares (and lock-blocks) the SBUF port pair DVE's 2x mode needs.
   * the result bytes {0,1,2,3} are read as fp8e4 DENORMALS (exact
     values c*2^-9) and multiplied against bf16-stationary x_q on the
     PE; the skinny (M=8) matmuls are packed 4-wide into the 128x128 PE
     via column tiling (tile_position=(0,32g)), 8 chunks of 448 outs.
   * bias and the code-minus-one correction are folded into PSUM by a
     rank-2 bf16 init matmul (acc_init = bias/(512*gamma) - Sx/512), so
     the epilogue is one ACT-engine scale out = acc*512*gamma per chunk,
     DMA'd out on the ACT HWDGE ring (SP ring stays weight-only).
   * optional WARM dummy matmuls fill the PE's DVE-wait micro-gaps so
     the HAM clock gate keeps the PE at 2.4 GHz.
"""

import os
import sys

sys.path.insert(0, "/opt/trn_rl_repo")

import numpy as np

import concourse.bacc as bacc
import concourse.mybir as mybir
from concourse import tile
from concourse.bass_utils import run_bass_kernel_spmd

AluOp = mybir.AluOpType
dt = mybir.dt

O, I, B = 28672, 8192, 8
NCORES = 8
OS = O // NCORES          # 3584 out-features per core
J = I // 4                # 2048 packed bytes per out-feature
NJT = J // 128            # 16 j-tiles

CH = 448                  # o-chunk size: 8 chunks, 2 per PE column group
NG = 4                    # PE column groups

CJT = int(os.environ.get("BITNET_CJT", "2"))       # j-tiles per DMA chunk
assert NJT % CJT == 0
WARM = int(os.environ.get("BITNET_WARM", "1"))     # PE warm-keeper MMs per k-step
PREWARM = int(os.environ.get("BITNET_PREWARM", "0"))  # PE warm-up MMs at body start
PLBUFS = int(os.environ.get("BITNET_PLBUFS", "4"))    # plane-tile buffers per tag

_cache = {}
LAST_RESULTS = None       # test harness can inspect profiling info here


def _build(repeat=1, mode="full"):
    # mode: "full" = real kernel; "dma" = weight stream only;
    #       "planes" = stream + unpack; "mm" = stream + matmul (no unpack)
    nc = bacc.Bacc("TRN2", target_bir_lowering=False, debug=False)

    pt = nc.dram_tensor("pt", [(NJT // CJT) * 128, CJT * OS], dt.uint8,
                        kind="ExternalInput")
    xq_d = nc.dram_tensor("xq", [128, 512], dt.bfloat16, kind="ExternalInput")
    biasb = nc.dram_tensor("biasb", [2, OS], dt.bfloat16, kind="ExternalInput")
    initl_d = nc.dram_tensor("initl", [2, 8], dt.bfloat16, kind="ExternalInput")
    g512_d = nc.dram_tensor("g512", [128, 1], dt.float32, kind="ExternalInput")
    out = nc.dram_tensor("out", [8, OS], dt.float32, kind="ExternalOutput")

    with tile.TileContext(nc) as tc:
        with (
            tc.tile_pool(name="io", bufs=2) as io,
            tc.tile_pool(name="wpool", bufs=3) as wpool,
            tc.tile_pool(name="plpool", bufs=PLBUFS) as plpool,
            tc.tile_pool(name="opool", bufs=4) as opool,
            tc.tile_pool(name="ps", bufs=1, space="PSUM") as ps,
        ):
            xq_t = io.tile([128, 512], dt.bfloat16)
            nc.sync.dma_start(xq_t[:], xq_d[:])
            biasb_t = io.tile([2, OS], dt.bfloat16)
            nc.sync.dma_start(biasb_t[:], biasb[:])
            initl = io.tile([2, 8], dt.bfloat16)
            nc.sync.dma_start(initl[:], initl_d[:])
            g512 = io.tile([128, 1], dt.float32)
            nc.sync.dma_start(g512[:], g512_d[:])

            if mode == "mm":
                # stream + matmul on a FIXED plane tile (no unpack): isolates
                # the PE rate (incl. HAM state) under the real loop structure
                pkf = io.tile([128, OS], dt.uint8)
                nc.vector.memset(pkf[:], 0)
                pkf8 = pkf[:].bitcast(dt.float8e4)

                def mm_body():
                    acc = ps.tile([128, 1024], dt.float32)
                    for jc in range(NJT // CJT):
                        cb = wpool.tile([128, CJT * OS], dt.uint8, tag="cb")
                        nc.sync.dma_start(cb[:], pt[jc * 128:(jc + 1) * 128, :])
                        for k in range(4):
                            for jt2 in range(CJT):
                                jt = jc * CJT + jt2
                                lhsT = xq_t[:, (jt * 4 + k) * 8:(jt * 4 + k + 1) * 8]
                                first = jt == 0 and k == 0
                                last = (jt == NJT - 1 and k == 3)
                                for cc in range(2):
                                    for g in range(NG):
                                        m = 2 * g + cc
                                        nc.tensor.matmul(
                                            acc[32 * g:32 * g + 8,
                                                cc * 512:cc * 512 + CH],
                                            lhsT,
                                            pkf8[:, m * CH:(m + 1) * CH],
                                            start=first, stop=last,
                                            tile_position=(0, 32 * g),
                                        )
                    ot = opool.tile([128, CH], dt.float32, tag="ot")
                    nc.vector.tensor_copy(ot[0:8, :], acc[0:8, 0:CH])
                    nc.scalar.dma_start(out[:, 0:CH], ot[0:8, :])

                if repeat == 1:
                    mm_body()
                else:
                    with tc.For_i(0, repeat):
                        mm_body()

            if mode == "both":
                # DVE full unpack + PE full matmul load, but matmuls read a
                # FIXED tile (no DVE->PE dependency): separates dependency
                # stalls from engine-coexistence contention
                pkf = io.tile([128, OS], dt.uint8)
                nc.vector.memset(pkf[:], 0)
                pkf8 = pkf[:].bitcast(dt.float8e4)

                def both_body():
                    acc = ps.tile([128, 1024], dt.float32)
                    for jc in range(NJT // CJT):
                        cb = wpool.tile([128, CJT * OS], dt.uint8, tag="cb")
                        nc.sync.dma_start(cb[:], pt[jc * 128:(jc + 1) * 128, :])
                        cbi = cb[:].bitcast(dt.int32)
                        for k in range(4):
                            pk = plpool.tile([128, CJT * (OS // 4)], dt.int32,
                                             tag=f"pk{k}")
                            nc.vector.tensor_scalar(
                                out=pk[:], in0=cbi,
                                scalar1=2 * k, scalar2=0x03030303,
                                op0=AluOp.logical_shift_right,
                                op1=AluOp.bitwise_and,
                            )
                            for jt2 in range(CJT):
                                jt = jc * CJT + jt2
                                lhsT = xq_t[:, (jt * 4 + k) * 8:(jt * 4 + k + 1) * 8]
                                first = jt == 0 and k == 0
                                last = (jt == NJT - 1 and k == 3)
                                for cc in range(2):
                                    for g in range(NG):
                                        m = 2 * g + cc
                                        nc.tensor.matmul(
                                            acc[32 * g:32 * g + 8,
                                                cc * 512:cc * 512 + CH],
                                            lhsT,
                                            pkf8[:, m * CH:(m + 1) * CH],
                                            start=first, stop=last,
                                            tile_position=(0, 32 * g),
                                        )
                    ot = opool.tile([128, CH], dt.float32, tag="ot")
                    nc.vector.tensor_copy(ot[0:8, :], acc[0:8, 0:CH])
                    nc.scalar.dma_start(out[:, 0:CH], ot[0:8, :])

                if repeat == 1:
                    both_body()
                else:
                    with tc.For_i(0, repeat):
                        both_body()

            def bisect_body():
                for jc in range(NJT // CJT):
                    cb = wpool.tile([128, CJT * OS], dt.uint8, tag="cb")
                    nc.sync.dma_start(cb[:], pt[jc * 128:(jc + 1) * 128, :])
                    if mode == "dma":
                        continue
                    cbi = cb[:].bitcast(dt.int32)
                    for k in range(4):
                        pk = plpool.tile([128, CJT * (OS // 4)], dt.int32,
                                         tag=f"pk{k}")
                        nc.vector.tensor_scalar(
                            out=pk[:], in0=cbi,
                            scalar1=2 * k, scalar2=0x03030303,
                            op0=AluOp.logical_shift_right,
                            op1=AluOp.bitwise_and,
                        )

            if mode in ("dma", "planes"):
                zt = io.tile([8, OS], dt.float32)
                nc.vector.memset(zt[:], 0.0)
                if repeat == 1:
                    bisect_body()
                else:
                    with tc.For_i(0, repeat):
                        bisect_body()
                nc.sync.dma_start(out[:, :], zt[:])

            def full_body():
                # rank-2 PSUM init: acc_init[b,o] = bias[o]/(512g) - Sx[b]/512
                # via lhsT=[[1/(512g)]*8, [-Sx[b]/512]] (host), rhs=[bias[o];1]
                acc = ps.tile([128, 1024], dt.float32)  # 2 banks; chunk cc at cc*512
                if WARM or PREWARM:
                    dummy_ps = ps.tile([128, 512], dt.float32)
                    dm_r = dm_c[:].bitcast(dt.float8e4)
                for _w in range(PREWARM):
                    # no-dependency matmuls: spin the PE during the chunk-0 DMA
                    # wait so HAM is already at K=8/8 when real matmuls arrive
                    nc.tensor.matmul(
                        dummy_ps[0:8, 0:CH], dm_l[:], dm_r,
                        start=True, stop=True, skip_group_check=True,
                    )
                for cc in range(2):
                    for g in range(NG):
                        m = 2 * g + cc
                        nc.tensor.matmul(
                            acc[32 * g:32 * g + 8, cc * 512:cc * 512 + CH],
                            initl[:],
                            biasb_t[:, m * CH:(m + 1) * CH],
                            start=True, stop=False,
                            tile_position=(0, 32 * g),
                        )

                # ---------- main loop: stream weights, unpack, matmul ----------
                for jc in range(NJT // CJT):
                    cb = wpool.tile([128, CJT * OS], dt.uint8, tag="cb")
                    nc.sync.dma_start(cb[:], pt[jc * 128:(jc + 1) * 128, :])
                    cbi = cb[:].bitcast(dt.int32)          # [128, CJT*896]
                    for k in range(4):
                        # one whole-chunk plane-extract per k: (w>>2k)&0x03..
                        pk = plpool.tile([128, CJT * (OS // 4)], dt.int32,
                                         tag=f"pk{k}")
                        if k == 0:
                            nc.vector.tensor_scalar(
                                out=pk[:], in0=cbi,
                                scalar1=0x03030303, scalar2=None,
                                op0=AluOp.bitwise_and,
                            )
                        else:
                            nc.vector.tensor_scalar(
                                out=pk[:], in0=cbi,
                                scalar1=2 * k, scalar2=0x03030303,
                                op0=AluOp.logical_shift_right,
                                op1=AluOp.bitwise_and,
                            )
                        pk8 = pk[:].bitcast(dt.float8e4)   # bytes c -> c*2^-9
                        for jt2 in range(CJT):
                            jt = jc * CJT + jt2
                            lhsT = xq_t[:, (jt * 4 + k) * 8:(jt * 4 + k + 1) * 8]
                            last = (jt == NJT - 1 and k == 3)
                            for cc in range(2):
                                for g in range(NG):
                                    m = 2 * g + cc          # global o-chunk
                                    nc.tensor.matmul(
                                        acc[32 * g:32 * g + 8,
                                            cc * 512:cc * 512 + CH],
                                        lhsT,
                                        pk8[:, jt2 * OS + m * CH:
                                             jt2 * OS + (m + 1) * CH],
                                        start=False, stop=last,
                                        tile_position=(0, 32 * g),
                                    )
                        for _w in range(WARM):
                            nc.tensor.matmul(
                                dummy_ps[0:8, 0:CH], dm_l[:], dm_r,
                                start=True, stop=True, skip_group_check=True,
                            )

                # ---------- epilogue: out = acc * 512*gamma on ACT ----------
                for cc in range(2):
                    ot = opool.tile([128, CH], dt.float32, tag="ot")
                    for g in range(NG):
                        m = 2 * g + cc
                        sl = slice(32 * g, 32 * g + 8)
                        nc.scalar.mul(
                            ot[sl, :], acc[sl, cc * 512:cc * 512 + CH],
                            g512[sl, :],
                        )
                        nc.scalar.dma_start(out[:, m * CH:(m + 1) * CH], ot[sl, :])

            if mode == "full":
                if WARM or PREWARM:
                    # constant operands for PE warm-keeper matmuls (fill the
                    # PE's DVE-wait micro-gaps so HAM keeps the PE at 2.4 GHz)
                    dm_l = io.tile([128, 8], dt.bfloat16)
                    nc.vector.memset(dm_l[:], 0.0)
                    dm_c = io.tile([128, CH], dt.uint8)
                    nc.vector.memset(dm_c[:], 0)
                if repeat == 1:
                    full_body()
                else:
                    with tc.For_i(0, repeat):
                        full_body()

    nc.compile()
    return nc


def prep_in_maps(x, packed_weight, weight_scale, bias):
    # ---- activation quantization, exactly mirroring the reference in f32 ----
    absmax = np.float32(max(np.abs(x).max(), np.float32(1e-5)))
    act_scale = absmax / np.float32(127.0)
    xq = np.clip(np.round(x / act_scale), -128, 127).astype(np.float32)  # RNE
    sx = xq.sum(axis=1, dtype=np.float64).astype(np.float32)             # [B]
    gamma = np.float32(weight_scale.reshape(())) * act_scale
    g512 = np.float32(gamma * np.float32(512.0))

    bf16 = mybir.dt.np(dt.bfloat16)
    # x_q -> PE-stationary layout [p, (jt k b)], bf16 (integers <=127, exact)
    xq_np = np.ascontiguousarray(
        xq.reshape(B, NJT, 128, 4).transpose(2, 1, 3, 0)
    ).reshape(128, 512).astype(bf16)
    g512_np = np.broadcast_to(np.float32(g512), (128, 1)).copy()
    initl = np.empty((2, 8), dtype=bf16)
    initl[0, :] = np.float32(1.0) / g512          # 1/(512*gamma)
    initl[1, :] = (-sx / np.float32(512.0)).astype(bf16)

    in_maps = []
    for c in range(NCORES):
        sl = slice(c * OS, (c + 1) * OS)
        # [OS, J] int32 -> u8 -> [J, OS] -> chunk-interleave: DMA chunk jc is
        # the contiguous rows [jc*128, (jc+1)*128) of a [NJT//CJT*128, CJT*OS]
        # array whose row p holds j-rows {jc*CJT*128 + jt2*128 + p}.
        ptc = packed_weight[sl, :].astype(np.uint8).T          # [J, OS]
        ptc = np.ascontiguousarray(
            ptc.reshape(NJT // CJT, CJT, 128, OS).transpose(0, 2, 1, 3)
        ).reshape((NJT // CJT) * 128, CJT * OS)
        biasb = np.empty((2, OS), dtype=bf16)
        biasb[0, :] = bias[sl].astype(bf16)
        biasb[1, :] = np.ones((OS,), dtype=bf16)
        in_maps.append({"pt": ptc, "xq": xq_np, "biasb": biasb,
                        "initl": initl, "g512": g512_np})
    return in_maps


def kernel(x, packed_weight, weight_scale, bias):
    global LAST_RESULTS
    repeat = int(os.environ.get("BITNET_REPEAT", "1"))
    mode = os.environ.get("BITNET_MODE", "full")
    key = (repeat, mode)
    if key not in _cache:
        _cache[key] = _build(repeat, mode=mode)
    nc = _cache[key]

    x = np.asarray(x, dtype=np.float32)
    packed_weight = np.asarray(packed_weight, dtype=np.int32)
    weight_scale = np.asarray(weight_scale, dtype=np.float32)
    bias = np.asarray(bias, dtype=np.float32)

    in_maps = prep_in_maps(x, packed_weight, weight_scale, bias)

    res = run_bass_kernel_spmd(nc, in_maps, list(range(NCORES)))
    LAST_RESULTS = res
    return np.concatenate(
        [np.asarray(res.results[c]["out"]) for c in range(NCORES)], axis=1
    ).reshape(B, O)
